# revision 22
# baseline (speedup 1.0000x reference)
"""3-layer GAT on 8 trn2 NeuronCores (Bass/Tile).

Strategy (dst-node sharding):
- N padded to 50176 = 392 dst-blocks x 128; 49 blocks per core.
- Per layer, a node-feature table holds per-node rows [h bf16 | el f32 | er f32]
  (768 B rows for 256-dim layers, 256 B rows for the 64-dim layer-3 input).
- Dense phase is sharded: each core computes rows for its 6272 nodes (fp32
  matmuls on PE), then one AllGather replicates the table to every core.
- Edge phase per dst-block: dma_gather pulls h|el rows by src (two gathers,
  src < 32768 and src >= 32768, because gather indices are int16), a 256 B
  slice gather pulls el|er rows by dst from the local shard, then
  w = exp(leaky_relu(el+er)) and a bf16 one-hot Sel matmul segment-reduces
  [w | w*h] into PSUM over the block's edge tiles, yielding the softmax
  denominator and the weighted sum together.  out = (sum w*h)/(sum w), which
  equals the reference's softmax-normalized aggregation exactly (the max
  subtraction cancels; logits here are O(1) so exp is safe in f32).
- The next layer's dense matmul for the block's 128 nodes is interleaved
  right after each block epilogue so it hides inside the gather stream.
"""

import os
import sys

sys.path.insert(0, "/opt/trn_rl_repo")

PHASES = int(os.environ.get("GAT_PHASES", "99"))
EDGE_CUT = int(os.environ.get("GAT_EDGE_CUT", "99"))

import numpy as np
import ml_dtypes

import concourse.bass as bass
import concourse.tile as tile
import concourse.mybir as mybir
from concourse import bacc
from concourse.bass_utils import run_bass_kernel_spmd

bf16 = mybir.dt.bfloat16
f32 = mybir.dt.float32
i16 = mybir.dt.int16
AF = mybir.ActivationFunctionType
ALU = mybir.AluOpType

NCORES = 8
P = 128
SPLIT = 32768
NEG_SLOPE = 0.2
H = 4
F = 64
D = H * F  # 256
ROW = 384  # bf16 cols per 256-dim table row (h 0:256 | el f32 256:264 | er f32 264:272 | pad)
ROW3 = 128  # bf16 cols per 64-dim table row (h 0:64 | el f32 64:66 | er f32 66:68 | pad)


def _wrap_idx_blocks(arr):
    """[NBLK, K] int16 -> [128, NBLK*K//16] in dma_gather index layout
    (idx i of each block at partition i%16, col i//16; 16-row pattern tiled
    8x down the partitions)."""
    nblk, k = arr.shape
    a = arr.reshape(nblk, k // 16, 16).transpose(0, 2, 1)  # [NBLK, 16, K/16]
    a = np.tile(a, (1, 8, 1))  # [NBLK, 128, K/16]
    return np.ascontiguousarray(a.transpose(1, 0, 2).reshape(128, -1))


def _col_layout(arr):
    """[NBLK, T*128] -> [128, NBLK*T]: slot t*128+p of block b at
    (p, b*T + t) -- matches the gather tile layout."""
    nblk, tk = arr.shape
    t = tk // 128
    a = arr.reshape(nblk, t, 128).transpose(2, 0, 1)  # [128, NBLK, T]
    return np.ascontiguousarray(a.reshape(128, nblk * t))


def _block_diag(a):
    """[H, F] -> [H*F, H] with a[h] on block-column h."""
    h, f = a.shape
    out = np.zeros((h * f, h), np.float32)
    for i in range(h):
        out[i * f : (i + 1) * f, i] = a[i]
    return out


def kernel(feat, src, dst, W1, al1, ar1, b1, W2, al2, ar2, b2, W3, al3, ar3, b3):
    feat = np.asarray(feat, np.float32)
    src = np.asarray(src).astype(np.int64)
    dst = np.asarray(dst).astype(np.int64)
    params = [np.asarray(p, np.float32) for p in (W1, al1, ar1, b1, W2, al2, ar2, b2, W3, al3, ar3, b3)]
    W1, al1, ar1, b1, W2, al2, ar2, b2, W3, al3, ar3, b3 = params
    assert abs(b1).max() == 0 and abs(b2).max() == 0 and abs(b3).max() == 0, (
        "non-zero GAT biases not implemented"
    )

    N, DIN = feat.shape
    E = src.shape[0]
    nblk_raw = -(-N // P)
    NBLK = -(-nblk_raw // NCORES) * NCORES  # 392
    NPAD = NBLK * P  # 50176
    BPC = NBLK // NCORES  # 49
    SHARD = BPC * P  # 6272

    # ---- host: edge preprocessing ----
    blk = dst // P
    order = np.lexsort((src, blk))
    src_s = src[order]
    dloc_s = (dst - blk * P)[order]
    blk_s = blk[order]
    counts = np.bincount(blk_s, minlength=NBLK)
    bstart = np.zeros(NBLK + 1, np.int64)
    np.cumsum(counts, out=bstart[1:])

    nlo = np.empty(NBLK, np.int64)
    for b in range(NBLK):
        nlo[b] = np.searchsorted(src_s[bstart[b] : bstart[b + 1]], SPLIT)
    nhi = counts - nlo
    TLO = int(-(-nlo.max() // P))
    THI = int(-(-nhi.max() // P))
    T = TLO + THI
    K_LO, K_HI = TLO * P, THI * P

    lo_idx = np.zeros((NBLK, K_LO), np.int16)
    hi_idx = np.zeros((NBLK, K_HI), np.int16)
    dstloc = np.full((NBLK, T * P), -1.0, np.float32)
    erloc = np.zeros((NBLK, T * P), np.int16)
    for b in range(NBLK):
        s, e = bstart[b], bstart[b + 1]
        nl = int(nlo[b])
        nh = int(e - s - nl)
        ss = src_s[s:e]
        dd = dloc_s[s:e]
        lo_idx[b, :nl] = ss[:nl]
        hi_idx[b, :nh] = ss[nl:] - SPLIT
        dstloc[b, :nl] = dd[:nl]
        erloc[b, :nl] = dd[:nl]
        dstloc[b, K_LO : K_LO + nh] = dd[nl:]
        erloc[b, K_LO : K_LO + nh] = dd[nl:]

    # ---- host: weights ----
    def wall(W, al, ar):
        wel = W @ _block_diag(al)
        wer = W @ _block_diag(ar)
        return np.concatenate([W, wel, wer], axis=1).astype(np.float32)

    wall1 = wall(W1, al1, ar1)  # [DIN, 264]
    wall2 = wall(W2, al2, ar2)  # [256, 264]
    wall3 = wall(W3, al3, ar3)  # [256, 66]
    NW = D + 2 * H  # 264
    NW3 = F + 2  # 66

    featT = np.zeros((DIN, NPAD), np.float32)
    featT[:, :N] = feat.T

    iota_np = np.tile(np.arange(P, dtype=np.float32), (P, 1)).astype(ml_dtypes.bfloat16)
    idn_np = np.eye(P, dtype=np.float32)

    # ---- host: per-core const blob (single int16 tensor -> one DMA) ----
    def blob_for_core(c):
        b0, b1_ = c * BPC, (c + 1) * BPC
        fields = [
            iota_np.view(np.int16),  # 128 cols bf16
            idn_np.view(np.int16),  # 256 cols f32
            wall1.view(np.int16),  # [DIN, 528]
            wall2[0:P].view(np.int16),
            wall2[P : 2 * P].view(np.int16),
            wall3[0:P].view(np.int16),
            wall3[P : 2 * P].view(np.int16),
            _wrap_idx_blocks(lo_idx[b0:b1_]),
            _wrap_idx_blocks(hi_idx[b0:b1_]),
            _wrap_idx_blocks(erloc[b0:b1_]),
            _col_layout(dstloc[b0:b1_].astype(ml_dtypes.bfloat16).view(np.int16)),
        ]
        # pad DIN=128-row fields to 128 rows (all already 128 rows except walls
        # built from [DIN,...] with DIN=128 -- asserted below)
        for f_ in fields:
            assert f_.shape[0] == P, f_.shape
        blob = np.concatenate(fields, axis=1)
        if blob.shape[1] % 2:
            blob = np.concatenate([blob, np.zeros((P, 1), np.int16)], axis=1)
        return np.ascontiguousarray(blob)

    assert DIN == P, "layer-1 input dim must be 128"
    blob0 = blob_for_core(0)
    CB = blob0.shape[1]
    offs = {}
    o = 0
    for name, w in [
        ("iota", 128),
        ("idn", 256),
        ("wall1", 2 * NW),
        ("wall2k0", 2 * NW),
        ("wall2k1", 2 * NW),
        ("wall3k0", 2 * NW3),
        ("wall3k1", 2 * NW3),
        ("lo", BPC * K_LO // 16),
        ("hi", BPC * K_HI // 16),
        ("erloc", BPC * T * P // 16),
        ("dstloc", BPC * T),
    ]:
        offs[name] = o
        o += w
    assert o == CB or o + 1 == CB

    # ---- build program (identical for all cores; per-core data via inputs) ----
    nc = bacc.Bacc("TRN2", target_bir_lowering=False, debug=False, num_devices=NCORES)

    cblob_in = nc.dram_tensor("cblob", [P, CB], i16, kind="ExternalInput")
    featT_in = nc.dram_tensor("featT", [P, SHARD], f32, kind="ExternalInput")
    out_ext = nc.dram_tensor("out", [SHARD, F], f32, kind="ExternalOutput")

    tab1_sh = nc.dram_tensor("tab1_sh", [SHARD, ROW], bf16)
    tab2_sh = nc.dram_tensor("tab2_sh", [SHARD, ROW], bf16)
    tab3_sh = nc.dram_tensor("tab3_sh", [SHARD, ROW3], bf16)
    tab1 = nc.dram_tensor("tab1", [NPAD, ROW], bf16, addr_space="Shared")
    tab2 = nc.dram_tensor("tab2", [NPAD, ROW], bf16, addr_space="Shared")
    tab3 = nc.dram_tensor("tab3", [NPAD, ROW3], bf16, addr_space="Shared")

    rg = [list(range(NCORES))]

    with tile.TileContext(nc) as tc:
        with (
            tc.tile_pool(name="const", bufs=1) as cp,
            tc.tile_pool(name="work", bufs=2) as wp,
            tc.tile_pool(name="small", bufs=2) as sp,
            tc.tile_pool(name="psum", bufs=2, space="PSUM") as pp,
        ):
            cblob = cp.tile([P, CB], i16)
            nc.sync.dma_start(cblob[:], cblob_in[:])
            iota = cblob[:, offs["iota"] : offs["iota"] + 128].bitcast(bf16)
            idn = cblob[:, offs["idn"] : offs["idn"] + 256].bitcast(f32)
            wall1_t = cblob[:, offs["wall1"] : offs["wall1"] + 2 * NW].bitcast(f32)
            wall2_t = [
                cblob[:, offs[f"wall2k{k}"] : offs[f"wall2k{k}"] + 2 * NW].bitcast(f32)
                for k in range(2)
            ]
            wall3_t = [
                cblob[:, offs[f"wall3k{k}"] : offs[f"wall3k{k}"] + 2 * NW3].bitcast(f32)
                for k in range(2)
            ]

            klo_reg = nc.gpsimd.to_reg(K_LO)
            khi_reg = nc.gpsimd.to_reg(K_HI)
            ker_reg = nc.gpsimd.to_reg(T * P)

            def idx_ap(field, j, k16):
                off = offs[field] + j * k16
                return cblob[:, off : off + k16]

            def dense_write(x_ap, j, wall_k, nw, tab_shard, row_cols, hsz, first):
                """dense for 128 nodes of block j: rows [h bf16 | el er f32]
                written to tab_shard. x_ap: [128, 256] f32 node-major (SBUF),
                or None with `first` giving the layer-1 lhsT directly."""
                psd = pp.tile([P, NW], f32, tag="psd", space="PSUM")
                nk = len(wall_k)
                if first is not None:
                    nc.tensor.matmul(psd[:, :nw], first, wall_k[0][:, :nw], start=True, stop=True)
                else:
                    lhsT = sp.tile([P, 2, P], f32, tag="lhsT")
                    for k in range(nk):
                        ptr = pp.tile([P, P], f32, tag="ptr", space="PSUM")
                        nc.tensor.transpose(ptr[:], x_ap[:, k * P : (k + 1) * P], idn)
                        nc.vector.tensor_copy(lhsT[:, k, :], ptr[:])
                    for k in range(nk):
                        nc.tensor.matmul(
                            psd[:, :nw],
                            lhsT[:, k, :],
                            wall_k[k][:, :nw],
                            start=(k == 0),
                            stop=(k == nk - 1),
                        )
                row = sp.tile([P, row_cols], bf16, tag="row")
                nc.vector.tensor_copy(row[:, 0:hsz], psd[:, 0:hsz])
                nc.vector.tensor_copy(
                    row[:, hsz : hsz + 2 * (nw - hsz)].bitcast(f32),
                    psd[:, hsz:nw],
                )
                nc.sync.dma_start(tab_shard[j * P : (j + 1) * P, :], row[:])

            def dump_rows(tab_shard, row, hsz):
                """debug: write first 64 h-cols of each shard row to out_ext"""
                for j in range(BPC):
                    r = sp.tile([P, row], bf16, tag="dump")
                    nc.sync.dma_start(r[:], tab_shard[j * P : (j + 1) * P, :])
                    rf = sp.tile([P, F], f32, tag="dumpf")
                    nc.vector.tensor_copy(rf[:], r[:, 0:F])
                    nc.sync.dma_start(out_ext[j * P : (j + 1) * P, :], rf[:])

            # ---- dense layer 1 (sharded; lhsT = feat^T slices, K=128) ----
            for j in range(BPC):
                ft = sp.tile([P, P], f32, tag="ft")
                nc.sync.dma_start(ft[:], featT_in[:, j * P : (j + 1) * P])
                dense_write(None, j, [wall1_t], NW, tab1_sh, ROW, D, first=ft[:])

            if PHASES == 1:
                dump_rows(tab1_sh, ROW, D)

            if PHASES >= 2:
                nc.gpsimd.collective_compute(
                    "AllGather", ALU.bypass, replica_groups=rg, ins=[tab1_sh[:]], outs=[tab1[:]]
                )

            # ---- edge phase for one layer ----
            def edge_layer(tab_full, tab_shard, row, heads, hsz, nxt):
                """tab_full: AG'd table, tab_shard: local shard (er source),
                row: bf16 cols per table row, heads: H, hsz: h cols,
                nxt: (wall_k, nw, tab_shard_next, row_next, hsz_next) or
                'out' for the final layer."""
                nmsg = heads + hsz
                for j in range(BPC):
                    hx = wp.tile([P, T, row], bf16, tag="hx")
                    nc.gpsimd.dma_gather(
                        hx[:, 0:TLO, :],
                        tab_full[0:SPLIT],
                        idx_ap("lo", j, K_LO // 16),
                        K_LO,
                        klo_reg,
                        row,
                        elem_step=row,
                        single_packet=False,
                    )
                    nc.gpsimd.dma_gather(
                        hx[:, TLO:T, :],
                        tab_full[SPLIT:NPAD],
                        idx_ap("hi", j, K_HI // 16),
                        K_HI,
                        khi_reg,
                        row,
                        elem_step=row,
                        single_packet=False,
                    )
                    # el|er chunk by dst from the local shard (last 256 B of row)
                    er = wp.tile([P, T, 128], bf16, tag="er")
                    nc.gpsimd.dma_gather(
                        er[:],
                        tab_shard[j * P : (j + 1) * P, row - 128 : row],
                        idx_ap("erloc", j, T * P // 16),
                        T * P,
                        ker_reg,
                        128,
                        elem_step=row,
                        single_packet=False,
                    )
                    if EDGE_CUT == 1:
                        # dump gathered h cols 0:64 of tile 0
                        df = sp.tile([P, F], f32, tag="edump")
                        nc.vector.tensor_copy(df[:], hx[:, 0, 0:F])
                        nc.sync.dma_start(out_ext[j * P : (j + 1) * P, :], df[:])
                        continue
                    # e = el[src] + er[dst]; w = exp(lrelu(e))
                    el_src = hx[:, :, hsz : hsz + 2 * heads].bitcast(f32)
                    eroff = 128 - (row - hsz)  # er cols within the 256B chunk
                    er_dst = er[:, :, eroff + 2 * heads : eroff + 4 * heads].bitcast(f32)
                    e_t = sp.tile([P, T, heads], f32, tag="e_t")
                    nc.vector.tensor_tensor(out=e_t[:], in0=el_src, in1=er_dst, op=ALU.add)
                    lr = sp.tile([P, T, heads], f32, tag="lr")
                    nc.vector.tensor_scalar_mul(lr[:], e_t[:], NEG_SLOPE)
                    nc.vector.tensor_tensor(out=lr[:], in0=e_t[:], in1=lr[:], op=ALU.max)
                    msg = wp.tile([P, T, nmsg], bf16, tag="msg")
                    nc.scalar.activation(msg[:, :, 0:heads], lr[:], AF.Exp)
                    # wh = w * h
                    nc.vector.tensor_tensor(
                        out=msg[:, :, heads:nmsg],
                        in0=hx[:, :, 0:hsz],
                        in1=msg[:, :, 0:heads].unsqueeze(3).to_broadcast([P, T, heads, F]),
                        op=ALU.mult,
                    )
                    if EDGE_CUT == 2:
                        df = sp.tile([P, F], f32, tag="edump")
                        nc.vector.tensor_copy(df[:], msg[:, 0, heads : heads + F])
                        nc.sync.dma_start(out_ext[j * P : (j + 1) * P, :], df[:])
                        continue
                    if EDGE_CUT == 5:
                        tt = int(os.environ.get("GAT_DUMP_TILE", "0"))
                        df = sp.tile([P, F], f32, tag="edump")
                        nc.gpsimd.memset(df[:], 0.0)
                        nc.vector.tensor_copy(df[:, 0:heads], msg[:, tt, 0:heads])
                        nc.sync.dma_start(out_ext[j * P : (j + 1) * P, :], df[:])
                        continue
                    # Sel one-hot [e, d] and segment-reduce into PSUM
                    sel = wp.tile([P, T, P], bf16, tag="sel")
                    dl_off = offs["dstloc"] + j * T
                    nc.vector.tensor_tensor(
                        out=sel[:],
                        in0=cblob[:, dl_off : dl_off + T]
                        .bitcast(bf16)
                        .unsqueeze(2)
                        .to_broadcast([P, T, P]),
                        in1=iota.unsqueeze(1).to_broadcast([P, T, P]),
                        op=ALU.is_equal,
                    )
                    ps = pp.tile([P, nmsg], f32, tag="agg", space="PSUM")
                    for t in range(T):
                        nc.tensor.matmul(
                            ps[:],
                            sel[:, t, :],
                            msg[:, t, :],
                            start=(t == 0),
                            stop=(t == T - 1),
                        )
                    if EDGE_CUT == 3:
                        df = sp.tile([P, F], f32, tag="edump")
                        nc.vector.tensor_copy(df[:], ps[:, heads : heads + F])
                        nc.sync.dma_start(out_ext[j * P : (j + 1) * P, :], df[:])
                        continue
                    if EDGE_CUT == 4:
                        df = sp.tile([P, F], f32, tag="edump")
                        nc.gpsimd.memset(df[:], 0.0)
                        nc.vector.tensor_copy(df[:, 0:heads], ps[:, 0:heads])
                        nc.sync.dma_start(out_ext[j * P : (j + 1) * P, :], df[:])
                        continue
                    # epilogue: out = act(wh_sum / w_sum)
                    rcp = sp.tile([P, 2, heads], f32, tag="rcp")
                    nc.vector.tensor_scalar(
                        out=rcp[:, 0, :], in0=ps[:, 0:heads], scalar1=1e-30,
                        scalar2=None, op0=ALU.max,
                    )
                    nc.vector.reciprocal(rcp[:, 1, :], rcp[:, 0, :])
                    x_sb = sp.tile([P, hsz], f32, tag="x_sb")
                    nc.vector.tensor_tensor(
                        out=x_sb[:].rearrange("p (h f) -> p h f", h=heads),
                        in0=ps[:, heads:nmsg].rearrange("p (h f) -> p h f", h=heads),
                        in1=rcp[:, 1, :].unsqueeze(2).to_broadcast([P, heads, F]),
                        op=ALU.mult,
                    )
                    if nxt != "out":
                        nc.vector.tensor_scalar_max(x_sb[:], x_sb[:], 0.0)
                    if nxt == "out":
                        dcol = F * int(os.environ.get("GAT_DUMP_HEAD", "0")) if EDGE_CUT != 99 or PHASES < 5 else 0
                        nc.sync.dma_start(
                            out_ext[j * P : (j + 1) * P, :], x_sb[:, dcol : dcol + F]
                        )
                    else:
                        wall_k, nw, tab_sh_n, row_n, hsz_n = nxt
                        dense_write(x_sb[:], j, wall_k, nw, tab_sh_n, row_n, hsz_n, None)

            if PHASES == 2:
                dump_rows(tab1_sh, ROW, D)  # exercises AG1 via nothing; just terminate
            if PHASES == 3:
                edge_layer(tab1, tab1_sh, ROW, H, D, "out")
            if PHASES >= 4:
                edge_layer(tab1, tab1_sh, ROW, H, D, (wall2_t, NW, tab2_sh, ROW, D))
            if PHASES == 4:
                dump_rows(tab2_sh, ROW, D)
            if PHASES >= 5:
                nc.gpsimd.collective_compute(
                    "AllGather", ALU.bypass, replica_groups=rg, ins=[tab2_sh[:]], outs=[tab2[:]]
                )
                edge_layer(tab2, tab2_sh, ROW, H, D, (wall3_t, NW3, tab3_sh, ROW3, F))
                nc.gpsimd.collective_compute(
                    "AllGather", ALU.bypass, replica_groups=rg, ins=[tab3_sh[:]], outs=[tab3[:]]
                )
                edge_layer(tab3, tab3_sh, ROW3, 1, F, "out")

    nc.compile()

    in_maps = [
        {
            "cblob": blob_for_core(c),
            "featT": np.ascontiguousarray(featT[:, c * SHARD : (c + 1) * SHARD]),
        }
        for c in range(NCORES)
    ]
    trace = os.environ.get("GAT_TRACE", "0") == "1"
    if trace and "antenv.axon_hooks" not in sys.modules:
        import importlib.util

        _spec = importlib.util.spec_from_file_location(
            "antenv.axon_hooks", "/opt/trn_rl_repo/antenv/axon_hooks.py"
        )
        _mod = importlib.util.module_from_spec(_spec)
        _spec.loader.exec_module(_mod)
        sys.modules["antenv.axon_hooks"] = _mod
    res = run_bass_kernel_spmd(nc, in_maps, list(range(NCORES)), trace=trace)
    if trace:
        print(f"HW exec time: {res.exec_time_ns} ns")
        global LAST_RESULTS
        LAST_RESULTS = res
    out = np.concatenate([res.results[c]["out"] for c in range(NCORES)], axis=0)
    return np.ascontiguousarray(out[:N]).astype(np.float32)


# revision 30
# speedup vs baseline: 1.4731x; 1.4731x over previous
"""3-layer GAT on 8 trn2 NeuronCores (Bass/Tile).

Strategy (dst-node sharding):
- N padded to 50176 = 392 dst-blocks x 128; 49 blocks per core.
- Per layer, a node-feature table holds per-node rows [h bf16 | el f32 | er f32]
  (768 B rows for 256-dim layers, 256 B rows for the 64-dim layer-3 input).
- Dense phase is sharded: each core computes rows for its 6272 nodes (fp32
  matmuls on PE), then one AllGather replicates the table to every core.
- Edge phase per dst-block: dma_gather pulls h|el rows by src (two gathers,
  src < 32768 and src >= 32768, because gather indices are int16), a 256 B
  slice gather pulls el|er rows by dst from the local shard, then
  w = exp(leaky_relu(el+er)) and a bf16 one-hot Sel matmul segment-reduces
  [w | w*h] into PSUM over the block's edge tiles, yielding the softmax
  denominator and the weighted sum together.  out = (sum w*h)/(sum w), which
  equals the reference's softmax-normalized aggregation exactly (the max
  subtraction cancels; logits here are O(1) so exp is safe in f32).
- The next layer's dense matmul for the block's 128 nodes is interleaved
  right after each block epilogue so it hides inside the gather stream.
"""

import os
import sys

sys.path.insert(0, "/opt/trn_rl_repo")

PHASES = int(os.environ.get("GAT_PHASES", "99"))
EDGE_CUT = int(os.environ.get("GAT_EDGE_CUT", "99"))

import numpy as np
import ml_dtypes

import concourse.bass as bass
import concourse.tile as tile
import concourse.mybir as mybir
from concourse import bacc
from concourse.bass_utils import run_bass_kernel_spmd

bf16 = mybir.dt.bfloat16
f32 = mybir.dt.float32
i16 = mybir.dt.int16
AF = mybir.ActivationFunctionType
ALU = mybir.AluOpType

NCORES = 8
P = 128
SPLIT = 32768
NEG_SLOPE = 0.2
H = 4
F = 64
D = H * F  # 256
ROW = 384  # bf16 cols per 256-dim table row (h 0:256 | el f32 256:264 | er f32 264:272 | pad)
ROW3 = 128  # bf16 cols per 64-dim table row (h 0:64 | el f32 64:66 | er f32 66:68 | pad)


def _wrap_idx_blocks(arr):
    """[NBLK, K] int16 -> [128, NBLK*K//16] in dma_gather index layout
    (idx i of each block at partition i%16, col i//16; 16-row pattern tiled
    8x down the partitions)."""
    nblk, k = arr.shape
    a = arr.reshape(nblk, k // 16, 16).transpose(0, 2, 1)  # [NBLK, 16, K/16]
    a = np.tile(a, (1, 8, 1))  # [NBLK, 128, K/16]
    return np.ascontiguousarray(a.transpose(1, 0, 2).reshape(128, -1))


def _col_layout(arr):
    """[NBLK, T*128] -> [128, NBLK*T]: slot t*128+p of block b at
    (p, b*T + t) -- matches the gather tile layout."""
    nblk, tk = arr.shape
    t = tk // 128
    a = arr.reshape(nblk, t, 128).transpose(2, 0, 1)  # [128, NBLK, T]
    return np.ascontiguousarray(a.reshape(128, nblk * t))


def _block_diag(a):
    """[H, F] -> [H*F, H] with a[h] on block-column h."""
    h, f = a.shape
    out = np.zeros((h * f, h), np.float32)
    for i in range(h):
        out[i * f : (i + 1) * f, i] = a[i]
    return out


def kernel(feat, src, dst, W1, al1, ar1, b1, W2, al2, ar2, b2, W3, al3, ar3, b3):
    feat = np.asarray(feat, np.float32)
    src = np.asarray(src).astype(np.int64)
    dst = np.asarray(dst).astype(np.int64)
    params = [np.asarray(p, np.float32) for p in (W1, al1, ar1, b1, W2, al2, ar2, b2, W3, al3, ar3, b3)]
    W1, al1, ar1, b1, W2, al2, ar2, b2, W3, al3, ar3, b3 = params
    assert abs(b1).max() == 0 and abs(b2).max() == 0 and abs(b3).max() == 0, (
        "non-zero GAT biases not implemented"
    )

    N, DIN = feat.shape
    E = src.shape[0]
    nblk_raw = -(-N // P)
    NBLK = -(-nblk_raw // NCORES) * NCORES  # 392
    NPAD = NBLK * P  # 50176
    BPC = NBLK // NCORES  # 49
    SHARD = BPC * P  # 6272

    # ---- host: edge preprocessing ----
    blk = dst // P
    order = np.lexsort((src, blk))
    src_s = src[order]
    dloc_s = (dst - blk * P)[order]
    blk_s = blk[order]
    counts = np.bincount(blk_s, minlength=NBLK)
    bstart = np.zeros(NBLK + 1, np.int64)
    np.cumsum(counts, out=bstart[1:])

    nlo = np.empty(NBLK, np.int64)
    for b in range(NBLK):
        nlo[b] = np.searchsorted(src_s[bstart[b] : bstart[b + 1]], SPLIT)
    nhi = counts - nlo
    TLO = int(-(-nlo.max() // P))
    THI = int(-(-nhi.max() // P))
    T = TLO + THI
    K_LO, K_HI = TLO * P, THI * P

    lo_idx = np.zeros((NBLK, K_LO), np.int16)
    hi_idx = np.zeros((NBLK, K_HI), np.int16)
    dstloc = np.full((NBLK, T * P), -1.0, np.float32)
    erloc = np.zeros((NBLK, T * P), np.int16)
    for b in range(NBLK):
        s, e = bstart[b], bstart[b + 1]
        nl = int(nlo[b])
        nh = int(e - s - nl)
        ss = src_s[s:e]
        dd = dloc_s[s:e]
        lo_idx[b, :nl] = ss[:nl]
        hi_idx[b, :nh] = ss[nl:] - SPLIT
        dstloc[b, :nl] = dd[:nl]
        erloc[b, :nl] = dd[:nl]
        dstloc[b, K_LO : K_LO + nh] = dd[nl:]
        erloc[b, K_LO : K_LO + nh] = dd[nl:]

    # ---- host: weights ----
    def wall(W, al, ar):
        wel = W @ _block_diag(al)
        wer = W @ _block_diag(ar)
        return np.concatenate([W, wel, wer], axis=1).astype(np.float32)

    wall1 = wall(W1, al1, ar1)  # [DIN, 264]
    wall2 = wall(W2, al2, ar2)  # [256, 264]
    wall3 = wall(W3, al3, ar3)  # [256, 66]
    NW = D + 2 * H  # 264
    NW3 = F + 2  # 66

    featT = np.zeros((DIN, NPAD), np.float32)
    featT[:, :N] = feat.T

    iota_np = np.tile(np.arange(P, dtype=np.float32), (P, 1)).astype(ml_dtypes.bfloat16)
    idn_np = np.eye(P, dtype=np.float32)

    # ---- host: per-core const blob (single int16 tensor -> one DMA) ----
    def blob_for_core(c):
        b0, b1_ = c * BPC, (c + 1) * BPC
        fields = [
            iota_np.view(np.int16),  # 128 cols bf16
            idn_np.view(np.int16),  # 256 cols f32
            idn_np.astype(ml_dtypes.bfloat16).view(np.int16),  # 128 cols bf16
            wall1.view(np.int16),  # [DIN, 528]
            wall2[0:P].view(np.int16),
            wall2[P : 2 * P].view(np.int16),
            wall3[0:P].view(np.int16),
            wall3[P : 2 * P].view(np.int16),
            _wrap_idx_blocks(lo_idx[b0:b1_]),
            _wrap_idx_blocks(hi_idx[b0:b1_]),
            _wrap_idx_blocks(erloc[b0:b1_]),
            _col_layout(dstloc[b0:b1_].astype(ml_dtypes.bfloat16).view(np.int16)),
        ]
        # pad DIN=128-row fields to 128 rows (all already 128 rows except walls
        # built from [DIN,...] with DIN=128 -- asserted below)
        for f_ in fields:
            assert f_.shape[0] == P, f_.shape
        blob = np.concatenate(fields, axis=1)
        if blob.shape[1] % 2:
            blob = np.concatenate([blob, np.zeros((P, 1), np.int16)], axis=1)
        return np.ascontiguousarray(blob)

    assert DIN == P, "layer-1 input dim must be 128"
    blob0 = blob_for_core(0)
    CB = blob0.shape[1]
    offs = {}
    o = 0
    for name, w in [
        ("iota", 128),
        ("idn", 256),
        ("idnb", 128),
        ("wall1", 2 * NW),
        ("wall2k0", 2 * NW),
        ("wall2k1", 2 * NW),
        ("wall3k0", 2 * NW3),
        ("wall3k1", 2 * NW3),
        ("lo", BPC * K_LO // 16),
        ("hi", BPC * K_HI // 16),
        ("erloc", BPC * T * P // 16),
        ("dstloc", BPC * T),
    ]:
        offs[name] = o
        o += w
    assert o == CB or o + 1 == CB

    # ---- build program (identical for all cores; per-core data via inputs) ----
    nc = bacc.Bacc("TRN2", target_bir_lowering=False, debug=False, num_devices=NCORES)

    cblob_in = nc.dram_tensor("cblob", [P, CB], i16, kind="ExternalInput")
    featT_in = nc.dram_tensor("featT", [P, SHARD], f32, kind="ExternalInput")
    out_ext = nc.dram_tensor("out", [SHARD, F], f32, kind="ExternalOutput")

    tab1_sh = nc.dram_tensor("tab1_sh", [SHARD, ROW], bf16)
    tab2_sh = nc.dram_tensor("tab2_sh", [SHARD, ROW], bf16)
    tab3_sh = nc.dram_tensor("tab3_sh", [SHARD, ROW3], bf16)
    tab1 = nc.dram_tensor("tab1", [NPAD, ROW], bf16, addr_space="Shared")
    tab2 = nc.dram_tensor("tab2", [NPAD, ROW], bf16, addr_space="Shared")
    tab3 = nc.dram_tensor("tab3", [NPAD, ROW3], bf16, addr_space="Shared")

    rg = [list(range(NCORES))]

    with tile.TileContext(nc) as tc:
        with (
            tc.tile_pool(name="const", bufs=1) as cp,
            tc.tile_pool(name="work", bufs=2) as wp,
            tc.tile_pool(name="small", bufs=2) as sp,
            tc.tile_pool(name="psum", bufs=2, space="PSUM") as pp,
        ):
            cblob = cp.tile([P, CB], i16)
            nc.sync.dma_start(cblob[:], cblob_in[:])
            iota = cblob[:, offs["iota"] : offs["iota"] + 128].bitcast(bf16)
            idn = cblob[:, offs["idn"] : offs["idn"] + 256].bitcast(f32)
            idnb = cblob[:, offs["idnb"] : offs["idnb"] + 128].bitcast(bf16)
            wall1_t = cblob[:, offs["wall1"] : offs["wall1"] + 2 * NW].bitcast(f32)
            wall2_t = [
                cblob[:, offs[f"wall2k{k}"] : offs[f"wall2k{k}"] + 2 * NW].bitcast(f32)
                for k in range(2)
            ]
            wall3_t = [
                cblob[:, offs[f"wall3k{k}"] : offs[f"wall3k{k}"] + 2 * NW3].bitcast(f32)
                for k in range(2)
            ]

            klo_reg = nc.gpsimd.to_reg(K_LO)
            khi_reg = nc.gpsimd.to_reg(K_HI)
            ker_reg = nc.gpsimd.to_reg(T * P)

            def idx_ap(field, j, k16):
                off = offs[field] + j * k16
                return cblob[:, off : off + k16]

            def dense_write(x_ap, j, wall_k, nw, tab_shard, row_cols, hsz, first):
                """dense for 128 nodes of block j: rows [h bf16 | el er f32]
                written to tab_shard. x_ap: [128, 256] f32 node-major (SBUF),
                or None with `first` giving the layer-1 lhsT directly."""
                psd = pp.tile([P, NW], f32, tag="psd", space="PSUM")
                nk = len(wall_k)
                if first is not None:
                    nc.tensor.matmul(psd[:, :nw], first, wall_k[0][:, :nw], start=True, stop=True)
                else:
                    lhsT = sp.tile([P, 2, P], f32, tag="lhsT")
                    for k in range(nk):
                        ptr = pp.tile([P, P], f32, tag="ptr", space="PSUM")
                        nc.tensor.transpose(ptr[:], x_ap[:, k * P : (k + 1) * P], idn)
                        nc.vector.tensor_copy(lhsT[:, k, :], ptr[:])
                    for k in range(nk):
                        nc.tensor.matmul(
                            psd[:, :nw],
                            lhsT[:, k, :],
                            wall_k[k][:, :nw],
                            start=(k == 0),
                            stop=(k == nk - 1),
                        )
                row = sp.tile([P, row_cols], bf16, tag="row")
                nc.vector.tensor_copy(row[:, 0:hsz], psd[:, 0:hsz])
                nc.vector.tensor_copy(
                    row[:, hsz : hsz + 2 * (nw - hsz)].bitcast(f32),
                    psd[:, hsz:nw],
                )
                nc.sync.dma_start(tab_shard[j * P : (j + 1) * P, :], row[:])

            def dump_rows(tab_shard, row, hsz):
                """debug: write first 64 h-cols of each shard row to out_ext"""
                for j in range(BPC):
                    r = sp.tile([P, row], bf16, tag="dump")
                    nc.sync.dma_start(r[:], tab_shard[j * P : (j + 1) * P, :])
                    rf = sp.tile([P, F], f32, tag="dumpf")
                    nc.vector.tensor_copy(rf[:], r[:, 0:F])
                    nc.sync.dma_start(out_ext[j * P : (j + 1) * P, :], rf[:])

            # ---- dense layer 1 (sharded; lhsT = feat^T slices, K=128) ----
            for j in range(BPC):
                ft = sp.tile([P, P], f32, tag="ft")
                nc.sync.dma_start(ft[:], featT_in[:, j * P : (j + 1) * P])
                dense_write(None, j, [wall1_t], NW, tab1_sh, ROW, D, first=ft[:])

            if PHASES == 1:
                dump_rows(tab1_sh, ROW, D)

            if PHASES >= 2:
                nc.gpsimd.collective_compute(
                    "AllGather", ALU.bypass, replica_groups=rg, ins=[tab1_sh[:]], outs=[tab1[:]]
                )

            # ---- edge phase for one layer ----
            def edge_layer(tab_full, tab_shard, row, heads, hsz, nxt):
                """tab_full: AG'd table, tab_shard: local shard (er source),
                row: bf16 cols per table row, heads: H, hsz: h cols,
                nxt: (wall_k, nw, tab_shard_next, row_next, hsz_next) or
                'out' for the final layer."""
                nmsg = heads + hsz
                for j in range(BPC):
                    hx = wp.tile([P, T, row], bf16, tag="hx")
                    nc.gpsimd.dma_gather(
                        hx[:, 0:TLO, :],
                        tab_full[0:SPLIT],
                        idx_ap("lo", j, K_LO // 16),
                        K_LO,
                        klo_reg,
                        row,
                        elem_step=row,
                        single_packet=False,
                    )
                    nc.gpsimd.dma_gather(
                        hx[:, TLO:T, :],
                        tab_full[SPLIT:NPAD],
                        idx_ap("hi", j, K_HI // 16),
                        K_HI,
                        khi_reg,
                        row,
                        elem_step=row,
                        single_packet=False,
                    )
                    # er for the block's 128 dsts: direct strided load of the
                    # 256B [el|er] row chunk, cast er to bf16
                    erch = sp.tile([P, 128], bf16, tag="erch")
                    nc.sync.dma_start(
                        erch[:], tab_shard[j * P : (j + 1) * P, row - 128 : row]
                    )
                    eroff0 = 128 - (row - hsz)
                    er_blk = sp.tile([P, heads], bf16, tag="er_blk")
                    nc.scalar.activation(
                        er_blk[:],
                        erch[:, eroff0 + 2 * heads : eroff0 + 4 * heads].bitcast(f32),
                        AF.Copy,
                    )
                    if EDGE_CUT == 1:
                        # dump gathered h cols 0:64 of tile 0
                        df = sp.tile([P, F], f32, tag="edump")
                        nc.vector.tensor_copy(df[:], hx[:, 0, 0:F])
                        nc.sync.dma_start(out_ext[j * P : (j + 1) * P, :], df[:])
                        continue
                    # Sel one-hot [e, d] built first; its PE transpose expands
                    # er_blk to per-edge er via one tiny matmul per tile
                    sel = wp.tile([P, T, P], bf16, tag="sel")
                    dl_off = offs["dstloc"] + j * T
                    nc.vector.tensor_tensor(
                        out=sel[:],
                        in0=cblob[:, dl_off : dl_off + T]
                        .bitcast(bf16)
                        .unsqueeze(2)
                        .to_broadcast([P, T, P]),
                        in1=iota.unsqueeze(1).to_broadcast([P, T, P]),
                        op=ALU.is_equal,
                    )
                    er_ps = pp.tile([P, T * heads], f32, tag="erps", space="PSUM")
                    for t in range(T):
                        selT_ps = pp.tile([P, P], bf16, tag="ptr", space="PSUM")
                        nc.tensor.transpose(selT_ps[:], sel[:, t, :], idnb)
                        selT = sp.tile([P, P], bf16, tag="selT_sb")
                        nc.scalar.activation(selT[:], selT_ps[:], AF.Copy)
                        nc.tensor.matmul(
                            er_ps[:, t * heads : (t + 1) * heads],
                            selT[:],
                            er_blk[:],
                            start=True,
                            stop=True,
                        )
                    # e = el[src] + er[dst]; w = exp(lrelu(e))
                    el_src = hx[:, :, hsz : hsz + 2 * heads].bitcast(f32)
                    e_t = sp.tile([P, T, heads], f32, tag="e_t")
                    nc.vector.tensor_tensor(
                        out=e_t[:],
                        in0=el_src,
                        in1=er_ps[:].rearrange("p (t h) -> p t h", h=heads),
                        op=ALU.add,
                    )
                    lr = sp.tile([P, T, heads], f32, tag="lr")
                    nc.vector.tensor_scalar_mul(lr[:], e_t[:], NEG_SLOPE)
                    nc.vector.tensor_tensor(out=lr[:], in0=e_t[:], in1=lr[:], op=ALU.max)
                    msg = wp.tile([P, T, nmsg], bf16, tag="msg")
                    nc.scalar.activation(msg[:, :, 0:heads], lr[:], AF.Exp)
                    # wh = w * h
                    nc.vector.tensor_tensor(
                        out=msg[:, :, heads:nmsg],
                        in0=hx[:, :, 0:hsz],
                        in1=msg[:, :, 0:heads].unsqueeze(3).to_broadcast([P, T, heads, F]),
                        op=ALU.mult,
                    )
                    if EDGE_CUT == 2:
                        df = sp.tile([P, F], f32, tag="edump")
                        nc.vector.tensor_copy(df[:], msg[:, 0, heads : heads + F])
                        nc.sync.dma_start(out_ext[j * P : (j + 1) * P, :], df[:])
                        continue
                    if EDGE_CUT == 5:
                        tt = int(os.environ.get("GAT_DUMP_TILE", "0"))
                        df = sp.tile([P, F], f32, tag="edump")
                        nc.gpsimd.memset(df[:], 0.0)
                        nc.vector.tensor_copy(df[:, 0:heads], msg[:, tt, 0:heads])
                        nc.sync.dma_start(out_ext[j * P : (j + 1) * P, :], df[:])
                        continue
                    # segment-reduce into PSUM
                    ps = pp.tile([P, nmsg], f32, tag="agg", space="PSUM")
                    for t in range(T):
                        nc.tensor.matmul(
                            ps[:],
                            sel[:, t, :],
                            msg[:, t, :],
                            start=(t == 0),
                            stop=(t == T - 1),
                        )
                    if EDGE_CUT == 3:
                        df = sp.tile([P, F], f32, tag="edump")
                        nc.vector.tensor_copy(df[:], ps[:, heads : heads + F])
                        nc.sync.dma_start(out_ext[j * P : (j + 1) * P, :], df[:])
                        continue
                    if EDGE_CUT == 4:
                        df = sp.tile([P, F], f32, tag="edump")
                        nc.gpsimd.memset(df[:], 0.0)
                        nc.vector.tensor_copy(df[:, 0:heads], ps[:, 0:heads])
                        nc.sync.dma_start(out_ext[j * P : (j + 1) * P, :], df[:])
                        continue
                    # epilogue: out = act(wh_sum / w_sum)
                    rcp = sp.tile([P, 2, heads], f32, tag="rcp")
                    nc.vector.tensor_scalar(
                        out=rcp[:, 0, :], in0=ps[:, 0:heads], scalar1=1e-30,
                        scalar2=None, op0=ALU.max,
                    )
                    nc.vector.reciprocal(rcp[:, 1, :], rcp[:, 0, :])
                    x_sb = sp.tile([P, hsz], f32, tag="x_sb")
                    nc.vector.tensor_tensor(
                        out=x_sb[:].rearrange("p (h f) -> p h f", h=heads),
                        in0=ps[:, heads:nmsg].rearrange("p (h f) -> p h f", h=heads),
                        in1=rcp[:, 1, :].unsqueeze(2).to_broadcast([P, heads, F]),
                        op=ALU.mult,
                    )
                    if nxt != "out":
                        nc.vector.tensor_scalar_max(x_sb[:], x_sb[:], 0.0)
                    if nxt == "out":
                        dcol = F * int(os.environ.get("GAT_DUMP_HEAD", "0")) if EDGE_CUT != 99 or PHASES < 5 else 0
                        nc.sync.dma_start(
                            out_ext[j * P : (j + 1) * P, :], x_sb[:, dcol : dcol + F]
                        )
                    else:
                        wall_k, nw, tab_sh_n, row_n, hsz_n = nxt
                        dense_write(x_sb[:], j, wall_k, nw, tab_sh_n, row_n, hsz_n, None)

            if PHASES == 2:
                dump_rows(tab1_sh, ROW, D)  # exercises AG1 via nothing; just terminate
            if PHASES == 3:
                edge_layer(tab1, tab1_sh, ROW, H, D, "out")
            if PHASES >= 4:
                edge_layer(tab1, tab1_sh, ROW, H, D, (wall2_t, NW, tab2_sh, ROW, D))
            if PHASES == 4:
                dump_rows(tab2_sh, ROW, D)
            if PHASES >= 5:
                nc.gpsimd.collective_compute(
                    "AllGather", ALU.bypass, replica_groups=rg, ins=[tab2_sh[:]], outs=[tab2[:]]
                )
                edge_layer(tab2, tab2_sh, ROW, H, D, (wall3_t, NW3, tab3_sh, ROW3, F))
                nc.gpsimd.collective_compute(
                    "AllGather", ALU.bypass, replica_groups=rg, ins=[tab3_sh[:]], outs=[tab3[:]]
                )
                edge_layer(tab3, tab3_sh, ROW3, 1, F, "out")

    nc.compile()

    in_maps = [
        {
            "cblob": blob_for_core(c),
            "featT": np.ascontiguousarray(featT[:, c * SHARD : (c + 1) * SHARD]),
        }
        for c in range(NCORES)
    ]
    trace = os.environ.get("GAT_TRACE", "0") == "1"
    if trace and "antenv.axon_hooks" not in sys.modules:
        import importlib.util

        _spec = importlib.util.spec_from_file_location(
            "antenv.axon_hooks", "/opt/trn_rl_repo/antenv/axon_hooks.py"
        )
        _mod = importlib.util.module_from_spec(_spec)
        _spec.loader.exec_module(_mod)
        sys.modules["antenv.axon_hooks"] = _mod
    res = run_bass_kernel_spmd(nc, in_maps, list(range(NCORES)), trace=trace)
    if trace:
        print(f"HW exec time: {res.exec_time_ns} ns")
        global LAST_RESULTS
        LAST_RESULTS = res
    out = np.concatenate([res.results[c]["out"] for c in range(NCORES)], axis=0)
    return np.ascontiguousarray(out[:N]).astype(np.float32)
